# revision 1
# baseline (speedup 1.0000x reference)
# Trainium2 Bass kernel for nn_Attention_81028853007030 — v2.
#
# Model: 1-unit LSTM over [B=64, L=2048, E=300] -> scores -> (buggy) mask ->
# softmax over L -> attn * x.
#
# v2 strategy (vs the v1 baseline):
#   - fp16 I/O: x is cast to fp16 on the host, out is written fp16 and
#     widened on the host. Halves HBM traffic (the roofline regime). End-to-
#     end error ~1e-3 absmax-relative, far under the 2e-2 gate.
#   - Wave-batched scan: 16 independent chains per partition (chunk len 8,
#     16-step warmup from zero; forget-gate decay bounds truncation error)
#     advance TOGETHER in each instruction, so a scan step is ~11 wide ops
#     instead of ~7 tiny ops per wave. g-gate tanh via 2*sigmoid(2x)-1 with
#     the 2x folded into W/b on the host, so one sigmoid covers all gates.
#   - xga layout [V, t(8), w2(18), 4]: slot (t, w2) = gates input for
#     timestep 8*(w2-2)+t of the partition's 128-step chunk; w2 in {0,1} is
#     the previous chunk's tail (DMA partition-shift), so warmup inputs are
#     plain strided reads, no copy pass.
#   - xg = x @ W_ih^T single fp16 matmul (no residual split; tolerance is
#     loose), PSUM->SBUF transpose copies round-robined over DVE/ACT/Pool,
#     gate results for a whole t-group (16 tau) batched into one PSUM bank
#     and flushed by one scalar_tensor_tensor (+bias).
#   - Both DMA streams run d-blocks [14, 15, 0..13] so the next iteration's
#     pass-0 (which needs the tail blocks) can start early in the For_i
#     steady state.

import numpy as np

B, L, E = 64, 2048, 300
NCORES = 8
S = B // NCORES          # sequences per core
V = 128                  # partitions = S * 16 chunks of 128 timesteps
W = 16                   # waves (independent scan chains per partition)
TCH = 128 // W           # chunk length per wave (8)
WM = 16                  # warmup steps
UW = WM + TCH            # scan steps (24)
ECH = [(0, 128), (128, 128), (256, 44)]  # E-chunks for the matmul
NEG = -1.0e30
DORD = [14, 15] + list(range(14))  # d-block order (tails first)

_CACHE = {}
DEBUG_TAPS = False  # emit xga / hs128 / attn as extra outputs


def _build_nc(loop_n=0):
    from contextlib import ExitStack

    import concourse.bacc as bacc
    import concourse.mybir as mybir
    from concourse import tile
    from concourse.masks import make_identity

    F32 = mybir.dt.float32
    F16 = mybir.dt.float16
    I32 = mybir.dt.int32
    Alu = mybir.AluOpType
    Act = mybir.ActivationFunctionType

    nc = bacc.Bacc("TRN2", target_bir_lowering=False, debug=False,
                   num_devices=NCORES)

    x_d = nc.dram_tensor("x", [S, L, E], F16, kind="ExternalInput")
    sl_d = nc.dram_tensor("sl", [S, 1], I32, kind="ExternalInput")
    wt_d = nc.dram_tensor("wt", [3, 128, 4], F16, kind="ExternalInput")
    cst_d = nc.dram_tensor("cst", [128, 80], F32, kind="ExternalInput")
    out_d = nc.dram_tensor("out", [S, L, E], F16, kind="ExternalOutput")
    if DEBUG_TAPS:
        dbg_xga = nc.dram_tensor("dbg_xga", [V, TCH * (W + 2) * 4], F32,
                                 kind="ExternalOutput")
        dbg_hs = nc.dram_tensor("dbg_hs", [V, 128], F32,
                                kind="ExternalOutput")
        dbg_at = nc.dram_tensor("dbg_at", [V, 128], F32,
                                kind="ExternalOutput")

    # [S, L, E] viewed as [(s k), t, e]; partition p = s*16 + k covers
    # L-rows k*128 .. k*128+127 of sequence s.
    x_v = x_d.ap().rearrange("s (k t) e -> (s k) t e", t=128)
    out_v = out_d.ap().rearrange("s (k t) e -> (s k) t e", t=128)

    with tile.TileContext(nc) as tc, ExitStack() as ctx:
        big = ctx.enter_context(tc.tile_pool(name="big", bufs=1))
        work = ctx.enter_context(tc.tile_pool(name="work", bufs=7))
        st = ctx.enter_context(tc.tile_pool(name="state", bufs=4))
        ppxt = ctx.enter_context(tc.tile_pool(name="ppxt", bufs=6, space="PSUM"))
        ppxg = ctx.enter_context(tc.tile_pool(name="ppxg", bufs=2, space="PSUM"))

        def emit_all():
            x_sb = big.tile([V, 128, E], F16, tag="x_sb")
            xga = big.tile([V, TCH, W + 2, 4], F32, tag="xga")
            hall = big.tile([V, UW, W], F32, tag="hall")
            ident = big.tile([128, 128], F16, tag="ident")
            wt_sb = big.tile([128, 3, 4], F16, tag="wt_sb")
            cst_sb = big.tile([128, 80], F32, tag="cst_sb")
            sl_sb = big.tile([S, 1], I32, tag="sl_sb")
            hseq = big.tile([S, L], F32, tag="hseq")
            attn_v = big.tile([V, 128], F32, tag="attn_v")
            zero16 = big.tile([V, W], F32, tag="zero16")

            # cst columns: [0:72] = b2 tiled 18x, [72:76] = w4 (w_hh permuted,
            # g scaled), [76:80] spare.
            bias18 = cst_sb[:, 0:72]
            w4c = cst_sb[:, 72:76]

            # ---- constants / setup ----
            make_identity(nc, ident[:])
            nc.sync.dma_start(wt_sb[:], wt_d.ap().rearrange("j p g -> p j g"))
            nc.sync.dma_start(cst_sb[:], cst_d.ap())
            nc.sync.dma_start(sl_sb[:], sl_d.ap())
            nc.vector.memset(zero16[:], 0.0)

            # ---- input DMA (tails first) ----
            for d in DORD:
                nc.sync.dma_start(x_sb[:, d * 8:(d + 1) * 8, :],
                                  x_v[:, d * 8:(d + 1) * 8, :])

            # ---- xg for one t-slot tau into PSUM quad ----
            cp_engines = [nc.vector, nc.scalar]  # GPSIMD cannot access PSUM

            def emit_T(tau):
                pst = ppxt.tile([128, 384], F16, tag="xtps")
                for j, (e0, cs) in enumerate(ECH):
                    nc.tensor.matmul(pst[0:cs, j * 128:(j + 1) * 128],
                                     lhsT=x_sb[:, tau, e0:e0 + cs],
                                     rhs=ident[:], is_transpose=True,
                                     start=True, stop=True)
                xt = work.tile([128, 384], F16, tag="xtsb")
                # rows 44:128 of the last chunk are garbage; never read.
                eng = cp_engines[tau % 2]
                if eng is nc.scalar:
                    nc.scalar.copy(out=xt[:], in_=pst[:])
                else:
                    eng.tensor_copy(out=xt[:], in_=pst[:])
                return xt

            def emit_G(xt, pg, q):
                for j, (e0, cs) in enumerate(ECH):
                    nc.tensor.matmul(pg[:, q * 4:(q + 1) * 4],
                                     lhsT=xt[0:cs, j * 128:(j + 1) * 128],
                                     rhs=wt_sb[0:cs, j, :],
                                     start=(j == 0), stop=(j == 2))

            # Software-pipeline PE: transposes run D tiles ahead of the gate
            # matmuls so the PSUM->SBUF copies drain while PE keeps working
            # (PE dispatches in order; a gate matmul waiting on its copy must
            # not starve the transposes queued behind it).
            DEPTH = 5

            def run_pass(taus, pg, qs):
                ring = []
                for i, tau in enumerate(taus):
                    ring.append(emit_T(tau))
                    if i >= DEPTH:
                        emit_G(ring[i - DEPTH], pg, qs[i - DEPTH])
                for i in range(max(0, len(taus) - DEPTH), len(taus)):
                    emit_G(ring[i], pg, qs[i])

            # ---- pass 0: tails (w = 14, 15 for every t) ----
            pg0 = ppxg.tile([V, 64], F32, tag="xgps")
            run_pass([(14 + j2) * TCH + t for t in range(TCH)
                      for j2 in range(2)],
                     pg0,
                     [t * 2 + j2 for t in range(TCH) for j2 in range(2)])
            nc.vector.scalar_tensor_tensor(xga[:, :, 16:18, :], in0=pg0[:],
                                           scalar=1.0, in1=bias18[:, 0:64],
                                           op0=Alu.mult, op1=Alu.add)

            # ---- warmup shift: previous chunk's tail -> w2 in {0, 1} ----
            nc.vector.memset(xga[:, :, 0:2, :], 0.0)
            for s in range(S):
                p0 = s * 16
                nc.sync.dma_start(xga[p0 + 1:p0 + 16, :, 0:2, :],
                                  xga[p0:p0 + 15, :, 16:18, :])

            # ---- scan step u (all 16 waves batched) ----
            c_prev = [None]

            def scan_step(u):
                t, a = u % TCH, u // TCH
                xsl = xga[:, t, a:a + 16, :]
                h_in = zero16[:] if u == 0 else hall[:, u - 1, :]
                p16 = st.tile([V, W, 4], F32, tag="p16")
                for g in range(4):
                    nc.vector.scalar_tensor_tensor(p16[:, :, g], in0=h_in,
                                                   scalar=w4c[:, g:g + 1],
                                                   in1=xsl[:, :, g],
                                                   op0=Alu.mult, op1=Alu.add)
                g16 = st.tile([V, W, 4], F32, tag="g16")
                nc.scalar.activation(g16[:], p16[:], Act.Sigmoid)
                gfx = st.tile([V, W], F32, tag="gfx")
                nc.vector.tensor_scalar(gfx[:], g16[:, :, 3], 2.0, -1.0,
                                        Alu.mult, Alu.add)
                ig = st.tile([V, W], F32, tag="ig")
                nc.vector.tensor_tensor(out=ig[:], in0=gfx[:],
                                        in1=g16[:, :, 0], op=Alu.mult)
                c_new = st.tile([V, W], F32, tag="c")
                if u == 0:
                    nc.vector.tensor_copy(out=c_new[:], in_=ig[:])
                else:
                    fc = st.tile([V, W], F32, tag="fc")
                    nc.vector.tensor_tensor(out=fc[:], in0=g16[:, :, 1],
                                            in1=c_prev[0][:], op=Alu.mult)
                    nc.vector.tensor_tensor(out=c_new[:], in0=fc[:],
                                            in1=ig[:], op=Alu.add)
                th = st.tile([V, W], F32, tag="th")
                nc.scalar.activation(th[:], c_new[:], Act.Tanh)
                nc.vector.tensor_tensor(out=hall[:, u, :], in0=th[:],
                                        in1=g16[:, :, 2], op=Alu.mult)
                c_prev[0] = c_new

            # ---- pass 1 groups (w = 0..13 per t) interleaved with scan ----
            for t in range(TCH):
                pg = ppxg.tile([V, 64], F32, tag="xgps")
                run_pass([w * TCH + t for w in range(14)], pg, list(range(14)))
                nc.vector.scalar_tensor_tensor(xga[:, t, 2:16, :],
                                               in0=pg[:, 0:56], scalar=1.0,
                                               in1=bias18[:, 0:56],
                                               op0=Alu.mult, op1=Alu.add)
                scan_step(t)
            for u in range(TCH, UW):
                scan_step(u)

            # ---- softmax over L per sequence ----
            hs128 = big.tile([V, 128], F32, tag="hs128")
            nc.vector.tensor_copy(
                out=hs128[:].rearrange("p (w t) -> p w t", t=TCH),
                in_=hall[:, WM:UW, :].rearrange("p t w -> p w t"))
            nc.sync.dma_start(hseq[:].rearrange("s (k t) -> s k t", t=128),
                              hs128[:])
            slf = big.tile([S, 1], F32, tag="slf")
            nc.vector.tensor_copy(out=slf[:], in_=sl_sb[:])
            cmp = big.tile([S, 1], F32, tag="cmp")
            nc.vector.tensor_scalar(cmp[:], slf[:], 0.0, None, Alu.is_gt)
            nc.vector.scalar_tensor_tensor(hseq[:, 0:1], in0=cmp[:],
                                           scalar=NEG, in1=hseq[:, 0:1],
                                           op0=Alu.mult, op1=Alu.add)
            # scores are in (-1, 1) (tanh-bounded), so exp cannot overflow
            # and the usual max-subtraction is unnecessary; the -1e30 masked
            # entry underflows to exp() = 0 exactly as with the shift.
            sume = big.tile([S, 1], F32, tag="sume")
            nc.scalar.activation(hseq[:], hseq[:], Act.Exp, bias=0.0,
                                 scale=1.0, accum_out=sume[:])
            rinv = big.tile([S, 1], F32, tag="rinv")
            nc.vector.reciprocal(rinv[:], sume[:])
            nc.vector.tensor_scalar_mul(hseq[:], hseq[:], rinv[:])
            nc.sync.dma_start(attn_v[:],
                              hseq[:].rearrange("s (k t) -> s k t", t=128))

            if DEBUG_TAPS:
                nc.sync.dma_start(
                    dbg_xga.ap(),
                    xga[:].rearrange("p t w g -> p (t w g)"))
                nc.sync.dma_start(dbg_hs.ap(), hs128[:])
                nc.sync.dma_start(dbg_at.ap(), attn_v[:])

            # ---- out = attn * x (in place, fp16), then DMA out ----
            for d in DORD:
                for tau in range(d * 8, (d + 1) * 8):
                    a = attn_v[:, tau:tau + 1]
                    xs = x_sb[:, tau, :]
                    if tau % 4 == 1:
                        nc.scalar.activation(xs, xs, Act.Copy, scale=a)
                    else:
                        nc.vector.tensor_scalar_mul(xs, xs, a)
                nc.sync.dma_start(out_v[:, d * 8:(d + 1) * 8, :],
                                  x_sb[:, d * 8:(d + 1) * 8, :])

        if loop_n:
            with tc.For_i(0, loop_n, 1):
                emit_all()
        else:
            emit_all()

    nc.compile()
    return nc


def _get_nc(loop_n=0):
    key = ("nc", loop_n, DEBUG_TAPS)
    if key not in _CACHE:
        _CACHE[key] = _build_nc(loop_n)
    return _CACHE[key]


# gate order i,f,g,o -> i,f,o,g  (g last so one sigmoid covers i,f,o and the
# 2x-scaled g column together)
_PERM = [0, 1, 3, 2]
_GSCL = np.array([1.0, 1.0, 1.0, 2.0], dtype=np.float64)


def make_in_maps(x, source_lengths, W_ih, W_hh, b_ih, b_hh):
    x16 = np.asarray(x, dtype=np.float16)
    sl = np.asarray(source_lengths).astype(np.int32).reshape(B, 1)
    wih = np.asarray(W_ih, dtype=np.float64)[_PERM] * _GSCL[:, None]
    w4 = np.asarray(W_hh, dtype=np.float64).reshape(4)[_PERM] * _GSCL
    b2 = (np.asarray(b_ih, dtype=np.float64)
          + np.asarray(b_hh, dtype=np.float64))[_PERM] * _GSCL

    wt = np.zeros((3, 128, 4), dtype=np.float16)
    wt.reshape(384, 4)[0:E] = wih.T.astype(np.float16)

    cst = np.zeros((128, 80), dtype=np.float32)
    cst[:, 0:72] = np.tile(b2.astype(np.float32), 18)
    cst[:, 72:76] = w4.astype(np.float32)

    in_maps = []
    for c in range(NCORES):
        in_maps.append({
            "x": np.ascontiguousarray(x16[c * S:(c + 1) * S]),
            "sl": np.ascontiguousarray(sl[c * S:(c + 1) * S]),
            "wt": wt,
            "cst": cst,
        })
    return in_maps


def kernel(x, source_lengths, W_ih, W_hh, b_ih, b_hh):
    from concourse.bass_utils import run_bass_kernel_spmd

    nc = _get_nc()
    in_maps = make_in_maps(x, source_lengths, W_ih, W_hh, b_ih, b_hh)
    res = run_bass_kernel_spmd(nc, in_maps, core_ids=list(range(NCORES)))
    out = np.concatenate(
        [res.results[c]["out"].astype(np.float32) for c in range(NCORES)],
        axis=0)
    return out

